# revision 5
# baseline (speedup 1.0000x reference)
import sys

for _p in ("/opt/trn_rl_repo", "/root/.axon_site/_ro/trn_rl_repo"):
    if _p not in sys.path:
        sys.path.insert(0, _p)

from contextlib import ExitStack

import ml_dtypes
import numpy as np

import concourse.bass as bass
import concourse.tile as tile
from concourse import bacc, mybir

BF16 = mybir.dt.bfloat16
F32 = mybir.dt.float32
AF = mybir.ActivationFunctionType
ALU = mybir.AluOpType
AX = mybir.AxisListType

N, K, D, E, H, O = 8192, 16, 256, 128, 512, 256
M_CORES = 8
P = 128
EPT = P * K
SQRT512 = float(np.sqrt(512.0).astype(np.float32))
INVS = 1.0 / SQRT512


def _build_program(n_tiles: int):
    nc = bacc.Bacc(None, target_bir_lowering=False)
    Nc = n_tiles * P
    NKc = Nc * K

    d_xT = nc.dram_tensor("xT", [D, Nc], BF16, kind="ExternalInput")
    d_ntT = nc.dram_tensor("ntT", [D, NKc], BF16, kind="ExternalInput")
    d_etT = nc.dram_tensor("etT", [E, NKc], BF16, kind="ExternalInput")
    d_nnd = nc.dram_tensor("nnd", [NKc, D], BF16, kind="ExternalInput")
    d_end = nc.dram_tensor("end", [NKc, E], BF16, kind="ExternalInput")
    d_pen = nc.dram_tensor("pen", [Nc, K], F32, kind="ExternalInput")
    d_w1xT = nc.dram_tensor("w1xT", [D, H], BF16, kind="ExternalInput")
    d_w2xT = nc.dram_tensor("w2xT", [H, H], BF16, kind="ExternalInput")
    d_w2n = nc.dram_tensor("w2n", [H, H], BF16, kind="ExternalInput")
    d_w2e = nc.dram_tensor("w2e", [H, H], BF16, kind="ExternalInput")
    d_w1nT = nc.dram_tensor("w1nT", [D, H], BF16, kind="ExternalInput")
    d_w1eT = nc.dram_tensor("w1eT", [E, H], BF16, kind="ExternalInput")
    d_wfxT = nc.dram_tensor("wfxT", [D, O], BF16, kind="ExternalInput")
    d_wfnT = nc.dram_tensor("wfnT", [D, O], BF16, kind="ExternalInput")
    d_wfeT = nc.dram_tensor("wfeT", [E, O], BF16, kind="ExternalInput")
    d_bfx = nc.dram_tensor("bfx", [P, 2], F32, kind="ExternalInput")
    d_bfn = nc.dram_tensor("bfn", [P, 2], F32, kind="ExternalInput")
    d_bfe = nc.dram_tensor("bfe", [P, 2], F32, kind="ExternalInput")
    d_bmask = nc.dram_tensor("bmask", [P, K, 8], BF16, kind="ExternalInput")
    d_out = nc.dram_tensor("outT", [3 * O, Nc], F32, kind="ExternalOutput")

    with tile.TileContext(nc) as tc, ExitStack() as ctx:
        singles = ctx.enter_context(tc.tile_pool(name="singles", bufs=1))
        work = ctx.enter_context(tc.tile_pool(name="work", bufs=2))
        wsbp = ctx.enter_context(tc.tile_pool(name="wsbp", bufs=2))
        small = ctx.enter_context(tc.tile_pool(name="small", bufs=3))
        dscr = ctx.enter_context(tc.tile_pool(name="dscr", bufs=2, space="DRAM"))
        psb = ctx.enter_context(tc.tile_pool(name="psb", bufs=3, space="PSUM"))
        psagg = ctx.enter_context(tc.tile_pool(name="psagg", bufs=3, space="PSUM"))
        psf = ctx.enter_context(tc.tile_pool(name="psf", bufs=2, space="PSUM"))

        def load_w(dram, kdim, mdim, name):
            kt = kdim // P
            t = singles.tile([P, kt, mdim], BF16, tag=name)
            for i in range(kt):
                nc.sync.dma_start(t[:, i, :], dram[i * P : (i + 1) * P, :])
            return t

        w1xT = load_w(d_w1xT, D, H, "w1xT")
        w2xT = load_w(d_w2xT, H, H, "w2xT")
        w2n = load_w(d_w2n, H, H, "w2n")
        w2e = load_w(d_w2e, H, H, "w2e")
        w1nT = load_w(d_w1nT, D, H, "w1nT")
        w1eT = load_w(d_w1eT, E, H, "w1eT")
        wfxT = load_w(d_wfxT, D, O, "wfxT")
        wfnT = load_w(d_wfnT, D, O, "wfnT")
        wfeT = load_w(d_wfeT, E, O, "wfeT")
        bfx = singles.tile([P, 2], F32, tag="bfx")
        nc.sync.dma_start(bfx, d_bfx[:, :])
        bfn = singles.tile([P, 2], F32, tag="bfn")
        nc.sync.dma_start(bfn, d_bfn[:, :])
        bfe = singles.tile([P, 2], F32, tag="bfe")
        nc.sync.dma_start(bfe, d_bfe[:, :])
        bmask = singles.tile([P, K, 8], BF16, tag="bmask")
        nc.sync.dma_start(bmask, d_bmask[:, :, :])

        ynT = singles.tile([P, 4, Nc], BF16, tag="ynT")
        yeT = singles.tile([P, 4, Nc], BF16, tag="yeT")

        with tc.tile_pool(name="p0tmp", bufs=1) as p0:
            xT = p0.tile([P, 2, Nc], BF16, tag="xT")
            for kd in range(2):
                nc.sync.dma_start(xT[:, kd, :], d_xT[kd * P : (kd + 1) * P, :])
            hx = p0.tile([P, 4, Nc], BF16, tag="hx")
            xatt = p0.tile([P, 4, Nc], BF16, tag="xatt")
            for c0 in range(0, Nc, 512):
                cw = min(512, Nc - c0)
                for mh in range(4):
                    ps = psb.tile([P, 512], F32, tag="psb")
                    for kd in range(2):
                        nc.tensor.matmul(
                            ps[:, :cw],
                            w1xT[:, kd, mh * P : (mh + 1) * P],
                            xT[:, kd, c0 : c0 + cw],
                            start=(kd == 0),
                            stop=(kd == 1),
                        )
                    nc.scalar.activation(hx[:, mh, c0 : c0 + cw], ps[:, :cw], AF.Tanh)
                for mh in range(4):
                    ps = psb.tile([P, 512], F32, tag="psb")
                    for kh in range(4):
                        nc.tensor.matmul(
                            ps[:, :cw],
                            w2xT[:, kh, mh * P : (mh + 1) * P],
                            hx[:, kh, c0 : c0 + cw],
                            start=(kh == 0),
                            stop=(kh == 3),
                        )
                    nc.vector.tensor_copy(xatt[:, mh, c0 : c0 + cw], ps[:, :cw])
                for dst, w in ((ynT, w2n), (yeT, w2e)):
                    for mh in range(4):
                        ps = psb.tile([P, 512], F32, tag="psb")
                        for kh in range(4):
                            nc.tensor.matmul(
                                ps[:, :cw],
                                w[:, kh, mh * P : (mh + 1) * P],
                                xatt[:, kh, c0 : c0 + cw],
                                start=(kh == 0),
                                stop=(kh == 3),
                            )
                        nc.vector.tensor_copy(dst[:, mh, c0 : c0 + cw], ps[:, :cw])
                for mo in range(2):
                    ps = psb.tile([P, 512], F32, tag="psb")
                    for kd in range(2):
                        nc.tensor.matmul(
                            ps[:, :cw],
                            wfxT[:, kd, mo * P : (mo + 1) * P],
                            xT[:, kd, c0 : c0 + cw],
                            start=(kd == 0),
                            stop=(kd == 1),
                        )
                    ob = small.tile([P, 512], F32, tag="fxout")
                    nc.scalar.activation(
                        ob[:, :cw], ps[:, :cw], AF.Relu, bias=bfx[:, mo : mo + 1]
                    )
                    nc.sync.dma_start(
                        d_out[mo * P : (mo + 1) * P, c0 : c0 + cw], ob[:, :cw]
                    )

        def attn_weights(t, yT, hT, scale, pen_sb, nm):
            wsb = wsbp.tile([P, EPT], BF16, tag="wsb")
            for c in range(4):
                ps = psb.tile([P, 512], F32, tag="psb")
                for kh in range(4):
                    nc.tensor.matmul(
                        ps,
                        yT[:, kh, t * P : (t + 1) * P],
                        hT[:, kh, c * 512 : (c + 1) * 512],
                        start=(kh == 0),
                        stop=(kh == 3),
                    )
                nc.vector.tensor_copy(wsb[:, c * 512 : (c + 1) * 512], ps)
            wsd = dscr.tile([P, EPT], BF16, tag="wsdram")
            nc.sync.dma_start(wsd, wsb)
            diag = small.tile([P, K], BF16, tag="diag" + nm)
            b = wsd[:, :]
            nc.sync.dma_start(
                diag,
                bass.AP(tensor=b.tensor, offset=b.offset, ap=[[EPT + K, P], [1, K]]),
            )
            if pen_sb is not None:
                logits = small.tile([P, K], F32, tag="logit" + nm)
                nc.vector.tensor_add(logits, diag, pen_sb)
            else:
                logits = diag
            mx = small.tile([P, 1], F32, tag="mx" + nm)
            nc.vector.tensor_reduce(mx, logits, axis=AX.X, op=ALU.max)
            nmx = small.tile([P, 1], F32, tag="nmx" + nm)
            nc.vector.tensor_scalar_mul(nmx, mx, -scale)
            et = small.tile([P, K], F32, tag="et" + nm)
            ssum = small.tile([P, 1], F32, tag="ssum" + nm)
            nc.scalar.activation(
                et, logits, AF.Exp, bias=nmx, scale=scale, accum_out=ssum
            )
            rc = small.tile([P, 1], F32, tag="rc" + nm)
            nc.vector.reciprocal(rc, ssum)
            wt = small.tile([P, K], BF16, tag="wt" + nm)
            nc.vector.tensor_scalar_mul(wt, et, rc)
            wdr = dscr.tile([P, K], BF16, tag="wdr" + nm)
            nc.sync.dma_start(wdr, wt)
            wcol = small.tile([P, K, 1], BF16, tag="wcol" + nm)
            b2 = wdr[:, :]
            nc.sync.dma_start(
                wcol[:, :, 0],
                bass.AP(tensor=b2.tensor, offset=b2.offset, ap=[[1, P], [P, K]]),
            )
            A = small.tile([P, K, 8], BF16, tag="A" + nm)
            nc.vector.tensor_mul(A, bmask, wcol.to_broadcast([P, K, 8]))
            return A

        for t in range(n_tiles):
            e0 = t * EPT
            ntT = work.tile([P, 2, EPT], BF16, tag="ntT")
            for kd in range(2):
                nc.sync.dma_start(
                    ntT[:, kd, :], d_ntT[kd * P : (kd + 1) * P, e0 : e0 + EPT]
                )
            etT = work.tile([P, EPT], BF16, tag="etT")
            nc.sync.dma_start(etT, d_etT[:, e0 : e0 + EPT])
            nnd = work.tile([P, K, D], BF16, tag="nnd")
            nc.sync.dma_start(
                nnd, d_nnd[e0 : e0 + EPT, :].rearrange("(g p) d -> p g d", p=P)
            )
            end = work.tile([P, K, E], BF16, tag="end")
            nc.sync.dma_start(
                end, d_end[e0 : e0 + EPT, :].rearrange("(g p) d -> p g d", p=P)
            )
            pen_sb = small.tile([P, K], F32, tag="pen")
            nc.sync.dma_start(pen_sb, d_pen[t * P : (t + 1) * P, :])

            hnT = work.tile([P, 4, EPT], BF16, tag="hnT")
            for c in range(4):
                for mh in range(4):
                    ps = psb.tile([P, 512], F32, tag="psb")
                    for kd in range(2):
                        nc.tensor.matmul(
                            ps,
                            w1nT[:, kd, mh * P : (mh + 1) * P],
                            ntT[:, kd, c * 512 : (c + 1) * 512],
                            start=(kd == 0),
                            stop=(kd == 1),
                        )
                    nc.scalar.activation(
                        hnT[:, mh, c * 512 : (c + 1) * 512], ps, AF.Tanh
                    )
            heT = work.tile([P, 4, EPT], BF16, tag="heT")
            for c in range(4):
                for mh in range(4):
                    ps = psb.tile([P, 512], F32, tag="psb")
                    nc.tensor.matmul(
                        ps,
                        w1eT[:, 0, mh * P : (mh + 1) * P],
                        etT[:, c * 512 : (c + 1) * 512],
                        start=True,
                        stop=True,
                    )
                    nc.scalar.activation(
                        heT[:, mh, c * 512 : (c + 1) * 512], ps, AF.Tanh
                    )

            An = attn_weights(t, ynT, hnT, INVS, None, "n")
            Ae = attn_weights(t, yeT, heT, 1.0, pen_sb, "e")

            agn0 = psagg.tile([P, P], F32, tag="psagg")
            agn1 = psagg.tile([P, P], F32, tag="psagg")
            age = psagg.tile([P, P], F32, tag="psagg")
            nc.vector.memset(agn0, 0.0)
            nc.vector.memset(agn1, 0.0)
            nc.vector.memset(age, 0.0)
            for g in range(K):
                nc.tensor.matmul(
                    agn0[:, g * 8 : (g + 1) * 8],
                    nnd[:, g, 0:P],
                    An[:, g, :],
                    start=False,
                    stop=(g == K - 1),
                    skip_group_check=True,
                )
                nc.tensor.matmul(
                    agn1[:, g * 8 : (g + 1) * 8],
                    nnd[:, g, P:D],
                    An[:, g, :],
                    start=False,
                    stop=(g == K - 1),
                    skip_group_check=True,
                )
                nc.tensor.matmul(
                    age[:, g * 8 : (g + 1) * 8],
                    end[:, g, :],
                    Ae[:, g, :],
                    start=False,
                    stop=(g == K - 1),
                    skip_group_check=True,
                )
            agn_sb = small.tile([P, 2, P], BF16, tag="agn")
            nc.vector.tensor_copy(agn_sb[:, 0, :], agn0)
            nc.vector.tensor_copy(agn_sb[:, 1, :], agn1)
            age_sb = small.tile([P, P], BF16, tag="age")
            nc.vector.tensor_copy(age_sb, age)

            for mo in range(2):
                ps = psf.tile([P, P], F32, tag="psf")
                for kd in range(2):
                    nc.tensor.matmul(
                        ps,
                        wfnT[:, kd, mo * P : (mo + 1) * P],
                        agn_sb[:, kd, :],
                        start=(kd == 0),
                        stop=(kd == 1),
                    )
                ob = small.tile([P, P], F32, tag="fout")
                nc.scalar.activation(ob, ps, AF.Relu, bias=bfn[:, mo : mo + 1])
                nc.sync.dma_start(
                    d_out[O + mo * P : O + (mo + 1) * P, t * P : (t + 1) * P], ob
                )
            for mo in range(2):
                ps = psf.tile([P, P], F32, tag="psf")
                nc.tensor.matmul(
                    ps,
                    wfeT[:, 0, mo * P : (mo + 1) * P],
                    age_sb,
                    start=True,
                    stop=True,
                )
                ob = small.tile([P, P], F32, tag="fout")
                nc.scalar.activation(ob, ps, AF.Relu, bias=bfe[:, mo : mo + 1])
                nc.sync.dma_start(
                    d_out[2 * O + mo * P : 2 * O + (mo + 1) * P, t * P : (t + 1) * P],
                    ob,
                )
    nc.compile()
    return nc


_CACHE: dict = {}


def _get_program(n_tiles: int):
    if n_tiles not in _CACHE:
        _CACHE[n_tiles] = _build_program(n_tiles)
    return _CACHE[n_tiles]


def _bf(a):
    return np.ascontiguousarray(a).astype(ml_dtypes.bfloat16)


def _prep_host(x, neibs, edge_emb, mask, W1x, W2x, W1n, W2n, W1e, W2e,
               Wfx, bfx, Wfn, bfn, Wfe, bfe):
    x = np.asarray(x, np.float32)
    neibs = np.asarray(neibs, np.float32)
    edge_emb = np.asarray(edge_emb, np.float32)
    mask = np.asarray(mask)
    pen_full = (-9999999.0 * mask.astype(np.float32)).astype(np.float32)

    bm = np.tile(
        (np.arange(P)[:, None] // K == np.arange(8)[None, :]).astype(np.float32),
        (1, K),
    ).reshape(P, K, 8)

    shared = {
        "w1xT": _bf(W1x.T), "w2xT": _bf(W2x.T), "w2n": _bf(W2n), "w2e": _bf(W2e),
        "w1nT": _bf(W1n.T), "w1eT": _bf(W1e.T),
        "wfxT": _bf(Wfx.T), "wfnT": _bf(Wfn.T), "wfeT": _bf(Wfe.T),
        "bfx": np.asarray(bfx, np.float32).reshape(2, P).T.copy(),
        "bfn": np.asarray(bfn, np.float32).reshape(2, P).T.copy(),
        "bfe": np.asarray(bfe, np.float32).reshape(2, P).T.copy(),
        "bmask": _bf(bm),
    }
    xT = _bf(x.T)
    ntT = _bf(neibs.T)
    etT = _bf(edge_emb.T)
    nnd = _bf(neibs)
    end = _bf(edge_emb)
    Ncn = N // M_CORES
    NKcn = Ncn * K
    in_maps = []
    for c in range(M_CORES):
        m = dict(shared)
        m["xT"] = np.ascontiguousarray(xT[:, c * Ncn : (c + 1) * Ncn])
        m["ntT"] = np.ascontiguousarray(ntT[:, c * NKcn : (c + 1) * NKcn])
        m["etT"] = np.ascontiguousarray(etT[:, c * NKcn : (c + 1) * NKcn])
        m["nnd"] = np.ascontiguousarray(nnd[c * NKcn : (c + 1) * NKcn])
        m["end"] = np.ascontiguousarray(end[c * NKcn : (c + 1) * NKcn])
        m["pen"] = np.ascontiguousarray(pen_full[c * Ncn : (c + 1) * Ncn])
        in_maps.append(m)
    return in_maps


def _run(inputs: dict, trace: bool = False, tmpdir: str | None = None):
    from concourse.bass_utils import run_bass_kernel_spmd

    nc = _get_program(N // M_CORES // P)
    in_maps = _prep_host(**inputs)
    res = run_bass_kernel_spmd(
        nc, in_maps, core_ids=list(range(M_CORES)), trace=trace, tmpdir=tmpdir
    )
    outs = [res.results[c]["outT"] for c in range(M_CORES)]
    full = np.concatenate(outs, axis=1).T
    return np.ascontiguousarray(full.astype(np.float32)), res


def kernel(**inputs) -> np.ndarray:
    out, _ = _run(inputs, trace=False)
    return out
